# revision 11
# baseline (speedup 1.0000x reference)
"""Trainium2 Bass kernel for nn_Attention_16518444221223 (sparse_attention).

Strategy: data-parallel over batch (16 seqs -> 8 cores x 2 seqs). Per core a
flash-attention-style kernel that never materializes the [b,h,n,n] score
tensor in HBM:
  - x tiles are PE-transposed once; QKV projections run in fp32 on the PE.
  - q/k are stored transposed ([d, tok]) in bf16 with the 8 heads (d=16)
    packed at partition offsets {0,32,64,96} across two tiles (the PE
    requires K<=32 operands to sit at 32-aligned base partitions).
  - scores are computed transposed (S^T[j, i] chunks), so the key-position
    mask folds into the per-partition bias of the Exp activation:
        P^T = Exp(scale * S^T + bias[j]),  bias = 0 (keep) / -300 (masked)
    No max-subtraction is needed: scaled dots are within +-5 for this
    problem, exp() cannot overflow, and softmax is shift-invariant.
  - P^T is the bf16 *stationary* operand of the PV matmul (fast weight
    load), with V carrying an extra ones-column per head so the softmax
    denominator falls out of the same matmul.
  - normalize, transpose via PE, output-project in fp32, add bias, DMA out.
"""

import sys

sys.path.insert(0, "/opt/trn_rl_repo")

import numpy as np

B, N, DIM, H, D = 16, 1024, 128, 8, 16
NCORES = 8
BLOC = B // NCORES  # 2 sequences per core
SCALE = float(DIM) ** -0.5
MASK_BIAS = -300.0
NT = N // 128  # 8 token tiles per sequence

_cache = {}


def _build_program(reps=1):
    import concourse.mybir as mybir
    import concourse.tile as tile
    from concourse import bacc
    from concourse._compat import axon_active
    from concourse.masks import make_identity

    f32 = mybir.dt.float32
    bf16 = mybir.dt.bfloat16

    nc = bacc.Bacc(
        "TRN2",
        target_bir_lowering=False,
        debug=not axon_active(),
        num_devices=NCORES,
    )

    x = nc.dram_tensor("x", [BLOC, N, DIM], f32, kind="ExternalInput")
    wqA = nc.dram_tensor("wqA", [DIM, 128], f32, kind="ExternalInput")
    wqB = nc.dram_tensor("wqB", [DIM, 128], f32, kind="ExternalInput")
    wkA = nc.dram_tensor("wkA", [DIM, 128], f32, kind="ExternalInput")
    wkB = nc.dram_tensor("wkB", [DIM, 128], f32, kind="ExternalInput")
    wvp = nc.dram_tensor("wvp", [DIM, H * (D + 1)], f32, kind="ExternalInput")
    wout = nc.dram_tensor("wout", [DIM, DIM], f32, kind="ExternalInput")
    boutB = nc.dram_tensor("boutB", [128, DIM], f32, kind="ExternalInput")
    mb = nc.dram_tensor("mb", [BLOC, 128, NT], f32, kind="ExternalInput")
    out = nc.dram_tensor("out", [BLOC, N, DIM], f32, kind="ExternalOutput")

    with tile.TileContext(nc) as tc:
        with (
            tc.tile_pool(name="consts", bufs=1) as consts,
            tc.tile_pool(name="xin", bufs=2) as p_xin,
            tc.tile_pool(name="xT", bufs=2) as p_xT,
            tc.tile_pool(name="qk", bufs=2) as p_qk,
            tc.tile_pool(name="vp", bufs=2) as p_vp,
            tc.tile_pool(name="PT", bufs=2) as p_PT,
            tc.tile_pool(name="oall", bufs=2) as p_oall,
            tc.tile_pool(name="ep", bufs=3) as p_ep,
            tc.tile_pool(name="fin", bufs=2) as p_fin,
            tc.tile_pool(name="ps_dots", bufs=2, space="PSUM") as ps_dots,
            tc.tile_pool(name="ps_sm", bufs=2, space="PSUM") as ps_sm,
            tc.tile_pool(name="ps_o", bufs=2, space="PSUM") as ps_o,
        ):
            # ---- constants ----
            wqA_sb = consts.tile([128, 128], f32, tag="wqA")
            wqB_sb = consts.tile([128, 128], f32, tag="wqB")
            wkA_sb = consts.tile([128, 128], f32, tag="wkA")
            wkB_sb = consts.tile([128, 128], f32, tag="wkB")
            wvp_sb = consts.tile([128, H * (D + 1)], f32, tag="wvp")
            wout_sb = consts.tile([128, 128], f32, tag="wout")
            bout_sb = consts.tile([128, 128], f32, tag="boutB")
            mb_sb = consts.tile([128, BLOC, NT], f32, tag="mb")
            ident = consts.tile([128, 128], f32, tag="ident")

            nc.sync.dma_start(wqA_sb[:], wqA.ap())
            nc.sync.dma_start(wqB_sb[:], wqB.ap())
            nc.sync.dma_start(wkA_sb[:], wkA.ap())
            nc.sync.dma_start(wkB_sb[:], wkB.ap())
            nc.sync.dma_start(wvp_sb[:], wvp.ap())
            nc.sync.dma_start(wout_sb[:], wout.ap())
            nc.sync.dma_start(bout_sb[:], boutB.ap())
            nc.sync.dma_start(
                mb_sb[:], mb.ap().rearrange("b p t -> p b t")
            )
            make_identity(nc, ident[:])

            def emit_batch(b):
                # ---- load x[b], transpose tiles -> xT_b [dim, tok] ----
                xin = p_xin.tile([128, NT, 128], f32, tag="xin")
                nc.sync.dma_start(
                    xin[:], x.ap()[b].rearrange("(t p) d -> p t d", p=128)
                )
                xT = p_xT.tile([128, N], f32, tag="xT")
                for t in range(NT):
                    pst = ps_sm.tile([128, 512], f32, tag="ps_sm")
                    nc.tensor.transpose(pst[:, :128], xin[:, t, :], ident[:])
                    nc.vector.tensor_copy(
                        out=xT[:, t * 128 : (t + 1) * 128], in_=pst[:, :128]
                    )

                # ---- q/k projections (fp32 PE) -> bf16 transposed layouts ----
                qkT = {}
                for nm, w_sb in (
                    ("qA", wqA_sb),
                    ("qB", wqB_sb),
                    ("kA", wkA_sb),
                    ("kB", wkB_sb),
                ):
                    dst = p_qk.tile([128, N], bf16, tag=nm)
                    qkT[nm] = dst
                    for g in range(N // 512):
                        psq = ps_sm.tile([128, 512], f32, tag="ps_sm")
                        nc.tensor.matmul(
                            psq[:],
                            w_sb[:],
                            xT[:, g * 512 : (g + 1) * 512],
                            start=True,
                            stop=True,
                        )
                        nc.vector.tensor_copy(
                            out=dst[:, g * 512 : (g + 1) * 512], in_=psq[:]
                        )

                # ---- v projection -> vp_b [128, chunk, head, 17] bf16 ----
                vp = p_vp.tile([128, NT, H, D + 1], bf16, tag="vp")
                for c in range(NT):
                    psv = ps_sm.tile([128, 512], f32, tag="ps_sm")
                    nc.tensor.matmul(
                        psv[:, : H * (D + 1)],
                        xT[:, c * 128 : (c + 1) * 128],
                        wvp_sb[:],
                        start=True,
                        stop=True,
                    )
                    nc.vector.tensor_copy(out=vp[:, c], in_=psv[:, : H * (D + 1)])
                    nc.vector.memset(vp[:, c, :, D], 1.0)

                # ---- attention per head ----
                o_all = p_oall.tile([128, NT, H, D + 1], f32, tag="oall")
                for h in range(H):
                    base = 32 * (h % 4)
                    sl = slice(base, base + D)
                    qT = qkT["qA"] if h < 4 else qkT["qB"]
                    kT = qkT["kA"] if h < 4 else qkT["kB"]
                    PT = p_PT.tile([128, NT, N], bf16, tag="PT")
                    for c in range(NT):
                        pss = ps_dots.tile([128, 1024], f32, tag="ps_dots")
                        for g in range(N // 512):
                            nc.tensor.matmul(
                                pss[:, g * 512 : (g + 1) * 512],
                                kT[sl, c * 128 : (c + 1) * 128],
                                qT[sl, g * 512 : (g + 1) * 512],
                                start=True,
                                stop=True,
                                tile_position=(base, 0),
                            )
                        nc.scalar.activation(
                            PT[:, c, :],
                            pss[:],
                            mybir.ActivationFunctionType.Exp,
                            bias=mb_sb[:, b, c : c + 1],
                            scale=SCALE,
                        )
                    for it in range(NT):
                        pso = ps_o.tile([128, D + 1], f32, tag="ps_o")
                        for c in range(NT):
                            nc.tensor.matmul(
                                pso[:],
                                PT[:, c, it * 128 : (it + 1) * 128],
                                vp[:, c, h, :],
                                start=(c == 0),
                                stop=(c == NT - 1),
                            )
                        nc.vector.tensor_copy(out=o_all[:, it, h, :], in_=pso[:])

                # ---- epilogue per token tile ----
                final = p_fin.tile([128, NT, 128], f32, tag="final")
                for it in range(NT):
                    recips = p_ep.tile([128, H], f32, tag="recips")
                    nc.vector.reciprocal(recips[:], o_all[:, it, :, D])
                    onorm = p_ep.tile([128, 128], f32, tag="onorm")
                    for h in range(H):
                        nc.vector.tensor_scalar_mul(
                            onorm[:, h * D : (h + 1) * D],
                            o_all[:, it, h, 0:D],
                            recips[:, h : h + 1],
                        )
                    pst2 = ps_sm.tile([128, 512], f32, tag="ps_sm")
                    nc.tensor.transpose(pst2[:, :128], onorm[:], ident[:])
                    onormT = p_ep.tile([128, 128], f32, tag="onormT")
                    nc.vector.tensor_copy(out=onormT[:], in_=pst2[:, :128])
                    psf = ps_sm.tile([128, 512], f32, tag="ps_sm")
                    nc.tensor.matmul(
                        psf[:, :128], onormT[:], wout_sb[:], start=True, stop=True
                    )
                    nc.vector.tensor_add(final[:, it, :], psf[:, :128], bout_sb[:])

                nc.sync.dma_start(
                    out.ap()[b].rearrange("(t p) d -> p t d", p=128), final[:]
                )

            if reps == 1:
                for b in range(BLOC):
                    emit_batch(b)
            else:
                # on-device loop: one dispatch runs the body `reps` times
                # (used for wall-clock-marginal timing measurements)
                with tc.For_i(0, reps, 1):
                    for b in range(BLOC):
                        emit_batch(b)

    nc.compile()
    return nc


def _get_program(reps=1):
    key = ("nc", reps)
    if key not in _cache:
        _cache[key] = _build_program(reps)
    return _cache[key]


def _host_prep(x, mask, maps, Wqkv, Wout, bout):
    """Build per-core input maps (weight repacking + mask bias precompute)."""
    x = np.ascontiguousarray(np.asarray(x, np.float32))
    Wqkv = np.asarray(Wqkv, np.float32)
    Wout = np.ascontiguousarray(np.asarray(Wout, np.float32))
    bout = np.asarray(bout, np.float32)
    Wq, Wk, Wv = Wqkv[:, :DIM], Wqkv[:, DIM : 2 * DIM], Wqkv[:, 2 * DIM :]

    def pack_qk(W, hs):
        out = np.zeros((DIM, 128), np.float32)
        for q, h in enumerate(hs):
            out[:, 32 * q : 32 * q + D] = W[:, D * h : D * (h + 1)]
        return out

    wqA = pack_qk(Wq, range(0, 4))
    wqB = pack_qk(Wq, range(4, 8))
    wkA = pack_qk(Wk, range(0, 4))
    wkB = pack_qk(Wk, range(4, 8))
    wvp = np.zeros((DIM, H * (D + 1)), np.float32)
    for h in range(H):
        wvp[:, (D + 1) * h : (D + 1) * h + D] = Wv[:, D * h : D * (h + 1)]
    boutB = np.broadcast_to(bout, (128, DIM)).copy()

    # combined key mask (block mask broadcasts over the full batch: B//K^2 == 1)
    m = np.concatenate([np.ones((1, 1), np.float32), np.asarray(mask, np.float32)], 1)
    mp = np.concatenate(
        [np.ones((B, 1), np.float32), np.asarray(maps, np.float32)], 1
    )
    keep = m * mp  # [B, N]
    mbias = ((keep - 1.0) * (-MASK_BIAS)).astype(np.float32)  # 0 / -300
    # [B, N] -> [B, chunk, 128] -> [B, 128, chunk]
    mbias = mbias.reshape(B, NT, 128).transpose(0, 2, 1).copy()

    in_maps = []
    for i in range(NCORES):
        in_maps.append(
            {
                "x": x[BLOC * i : BLOC * (i + 1)],
                "wqA": wqA,
                "wqB": wqB,
                "wkA": wkA,
                "wkB": wkB,
                "wvp": wvp,
                "wout": Wout,
                "boutB": boutB,
                "mb": np.ascontiguousarray(
                    mbias[BLOC * i : BLOC * (i + 1)]
                ),
            }
        )
    return in_maps


def kernel(x, mask, maps, Wqkv, Wout, bout, K):
    from concourse.bass_utils import run_bass_kernel_spmd

    nc = _get_program()
    in_maps = _host_prep(x, mask, maps, Wqkv, Wout, bout)
    res = run_bass_kernel_spmd(nc, in_maps, list(range(NCORES)))
    return np.concatenate(
        [res.results[i]["out"] for i in range(NCORES)], axis=0
    ).astype(np.float32)


# revision 29
# speedup vs baseline: 1.2114x; 1.2114x over previous
"""Trainium2 Bass kernel for nn_Attention_16518444221223 (sparse_attention).

Strategy: data-parallel over batch (16 seqs -> 8 cores x 2 seqs). Per core a
flash-attention-style kernel that never materializes the [b,h,n,n] score
tensor in HBM:
  - x tiles are PE-transposed once; QKV projections run in fp32 on the PE.
  - q/k are stored transposed ([d, tok]) in bf16 with the 8 heads (d=16)
    packed at partition offsets {0,32,64,96} across two tiles (the PE
    requires K<=32 operands to sit at 32-aligned base partitions).
  - scores are computed transposed (S^T[j, i] chunks), so the key-position
    mask folds into the per-partition bias of the Exp activation:
        P^T = Exp(scale * S^T + bias[j]),  bias = 0 (keep) / -300 (masked)
    No max-subtraction is needed: scaled dots are within +-5 for this
    problem, exp() cannot overflow, and softmax is shift-invariant.
  - P^T is the bf16 *stationary* operand of the PV matmul (fast weight
    load), with V carrying an extra ones-column per head so the softmax
    denominator falls out of the same matmul.
  - normalize, transpose via PE, output-project in fp32, add bias, DMA out.
"""

import sys

sys.path.insert(0, "/opt/trn_rl_repo")

import numpy as np

B, N, DIM, H, D = 16, 1024, 128, 8, 16
NCORES = 8
BLOC = B // NCORES  # 2 sequences per core
SCALE = float(DIM) ** -0.5
MASK_BIAS = -300.0
NT = N // 128  # 8 token tiles per sequence

_cache = {}
VARIANT = "E"  # "A": 2-bank dots psum + single [128,1024] exp per chunk
               # "B": 1-bank dots psum + two [128,512] exps per chunk
               # "D": like A, but attnV uses moving-P matmuls (8x fewer PE
               #      instructions; o comes out transposed, epilogue adjusts)
               # "E": phase-batched (both seqs prologue -> heads -> epilogues)
               #      with PV matmuls interleaved into the dots stream


def _build_program(reps=1, variant=None):
    if variant is None:
        variant = VARIANT
    import concourse.mybir as mybir
    import concourse.tile as tile
    from concourse import bacc
    from concourse._compat import axon_active
    from concourse.masks import make_identity

    f32 = mybir.dt.float32
    bf16 = mybir.dt.bfloat16

    nc = bacc.Bacc(
        "TRN2",
        target_bir_lowering=False,
        debug=not axon_active(),
        num_devices=NCORES,
    )

    x = nc.dram_tensor("x", [BLOC, N, DIM], f32, kind="ExternalInput")
    wqA = nc.dram_tensor("wqA", [DIM, 128], bf16, kind="ExternalInput")
    wqB = nc.dram_tensor("wqB", [DIM, 128], bf16, kind="ExternalInput")
    wkA = nc.dram_tensor("wkA", [DIM, 128], bf16, kind="ExternalInput")
    wkB = nc.dram_tensor("wkB", [DIM, 128], bf16, kind="ExternalInput")
    VPW = 32  # per-head V block width: 16 dims + 1 ones-col + 15 zero pad
    wvp = nc.dram_tensor("wvp", [DIM, H * VPW], bf16, kind="ExternalInput")
    wout = nc.dram_tensor("wout", [DIM, DIM], f32, kind="ExternalInput")
    boutB = nc.dram_tensor("boutB", [128, DIM], f32, kind="ExternalInput")
    mb = nc.dram_tensor("mb", [BLOC, 128, NT], f32, kind="ExternalInput")
    out = nc.dram_tensor("out", [BLOC, N, DIM], f32, kind="ExternalOutput")

    with tile.TileContext(nc) as tc:
        with (
            tc.tile_pool(name="consts", bufs=1) as consts,
            tc.tile_pool(name="xin", bufs=2) as p_xin,
            tc.tile_pool(name="xT", bufs=2) as p_xT,
            tc.tile_pool(name="qk", bufs=2) as p_qk,
            tc.tile_pool(name="vp", bufs=2) as p_vp,
            tc.tile_pool(name="PT", bufs=3) as p_PT,
            tc.tile_pool(name="oall", bufs=2) as p_oall,
            tc.tile_pool(name="ep", bufs=3) as p_ep,
            tc.tile_pool(name="fin", bufs=2) as p_fin,
            tc.tile_pool(
                name="ps_dots", bufs=(4 if variant == "B" else 2), space="PSUM"
            ) as ps_dots,
            tc.tile_pool(name="ps_sm", bufs=2, space="PSUM") as ps_sm,
            tc.tile_pool(name="ps_o", bufs=2, space="PSUM") as ps_o,
            tc.tile_pool(name="ps_oe", bufs=2, space="PSUM") as ps_oe,
        ):
            # PSUM banks: A: dots 2x2 + prologue 2 + (o/epilogue) 2 = 8
            #             B: dots 4x1 + prologue 2 + (o/epilogue) 2 = 8
            # ---- constants ----
            wqA_sb = consts.tile([128, 128], bf16, tag="wqA")
            wqB_sb = consts.tile([128, 128], bf16, tag="wqB")
            wkA_sb = consts.tile([128, 128], bf16, tag="wkA")
            wkB_sb = consts.tile([128, 128], bf16, tag="wkB")
            wvp_sb = consts.tile([128, H * VPW], bf16, tag="wvp")
            wout_sb = consts.tile([128, 128], f32, tag="wout")
            bout_sb = consts.tile([128, 128], f32, tag="boutB")
            mb_sb = consts.tile([128, BLOC, NT], f32, tag="mb")
            ident = consts.tile([128, 128], f32, tag="ident")

            nc.sync.dma_start(wqA_sb[:], wqA.ap())
            nc.sync.dma_start(wkA_sb[:], wkA.ap())
            nc.sync.dma_start(
                mb_sb[:], mb.ap().rearrange("b p t -> p b t")
            )
            nc.sync.dma_start(wqB_sb[:], wqB.ap())
            nc.sync.dma_start(wkB_sb[:], wkB.ap())
            nc.sync.dma_start(wvp_sb[:], wvp.ap())
            nc.sync.dma_start(wout_sb[:], wout.ap())
            nc.sync.dma_start(bout_sb[:], boutB.ap())
            make_identity(nc, ident[:])
            actwarm = consts.tile([128, 1], f32, tag="actwarm")
            nc.gpsimd.memset(actwarm[:], 0.0)
            nc.scalar.activation(
                actwarm[:], actwarm[:], mybir.ActivationFunctionType.Exp
            )

            def emit_batch(b):
                # ---- load x[b], transpose tiles -> xT_b [dim, tok] ----
                xin = p_xin.tile([128, NT, 128], f32, tag="xin")
                nc.sync.dma_start(
                    xin[:], x.ap()[b].rearrange("(t p) d -> p t d", p=128)
                )
                xT = p_xT.tile([128, N], bf16, tag="xT")
                for t in range(NT):
                    pst = ps_sm.tile([128, 512], f32, tag="ps_sm")
                    nc.tensor.transpose(pst[:, :128], xin[:, t, :], ident[:])
                    nc.vector.tensor_copy(
                        out=xT[:, t * 128 : (t + 1) * 128], in_=pst[:, :128]
                    )

                # ---- q/k projections (fp32 PE) -> bf16 transposed layouts ----
                qkT = {}
                for nm, w_sb in (
                    ("qA", wqA_sb),
                    ("kA", wkA_sb),
                    ("qB", wqB_sb),
                    ("kB", wkB_sb),
                ):
                    dst = p_qk.tile([128, N], bf16, tag=nm)
                    qkT[nm] = dst
                    for g in range(N // 512):
                        psq = ps_sm.tile([128, 512], f32, tag="ps_sm")
                        nc.tensor.matmul(
                            psq[:],
                            w_sb[:],
                            xT[:, g * 512 : (g + 1) * 512],
                            start=True,
                            stop=True,
                        )
                        nc.vector.tensor_copy(
                            out=dst[:, g * 512 : (g + 1) * 512], in_=psq[:]
                        )

                # ---- v projection -> vp_b [128, chunk, head, 17] bf16 ----
                vp = p_vp.tile([128, NT, H, VPW], bf16, tag="vp")
                for c in range(NT):
                    psv = ps_sm.tile([128, 512], f32, tag="ps_sm")
                    nc.tensor.matmul(
                        psv[:, : H * VPW],
                        xT[:, c * 128 : (c + 1) * 128],
                        wvp_sb[:],
                        start=True,
                        stop=True,
                    )
                    nc.vector.tensor_copy(out=vp[:, c], in_=psv[:, : H * VPW])
                    nc.gpsimd.memset(vp[:, c, :, D], 1.0)

                # ---- attention ----
                if variant == "D":
                    emit_attention_D(b, qkT, vp)
                    return
                o_all = p_oall.tile([128, NT, H, D + 1], f32, tag="oall")
                for h in range(H):
                    base = 32 * (h % 4)
                    sl = slice(base, base + D)
                    qT = qkT["qA"] if h < 4 else qkT["qB"]
                    kT = qkT["kA"] if h < 4 else qkT["kB"]
                    PT = p_PT.tile([128, NT, N], bf16, tag="PT")
                    for c in range(NT):
                        if variant == "A":
                            pss = ps_dots.tile([128, 1024], f32, tag="ps_dots")
                            for g in range(N // 512):
                                nc.tensor.matmul(
                                    pss[:, g * 512 : (g + 1) * 512],
                                    kT[sl, c * 128 : (c + 1) * 128],
                                    qT[sl, g * 512 : (g + 1) * 512],
                                    start=True,
                                    stop=True,
                                    tile_position=(base, 0),
                                )
                            nc.scalar.activation(
                                PT[:, c, :],
                                pss[:],
                                mybir.ActivationFunctionType.Exp,
                                bias=mb_sb[:, b, c : c + 1],
                                scale=SCALE,
                            )
                        else:
                            for g in range(N // 512):
                                pss = ps_dots.tile([128, 512], f32, tag="ps_dots")
                                nc.tensor.matmul(
                                    pss[:],
                                    kT[sl, c * 128 : (c + 1) * 128],
                                    qT[sl, g * 512 : (g + 1) * 512],
                                    start=True,
                                    stop=True,
                                    tile_position=(base, 0),
                                )
                                nc.scalar.activation(
                                    PT[:, c, g * 512 : (g + 1) * 512],
                                    pss[:],
                                    mybir.ActivationFunctionType.Exp,
                                    bias=mb_sb[:, b, c : c + 1],
                                    scale=SCALE,
                                )
                    for it4 in range(NT // 4):
                        o4 = ps_o.tile([128, 4, D + 1], f32, tag="ps_o", name="o4")
                        for itm in range(4):
                            it = it4 * 4 + itm
                            for c in range(NT):
                                nc.tensor.matmul(
                                    o4[:, itm, :],
                                    PT[:, c, it * 128 : (it + 1) * 128],
                                    vp[:, c, h, 0 : D + 1],
                                    start=(c == 0),
                                    stop=(c == NT - 1),
                                )
                        nc.vector.tensor_copy(
                            out=o_all[:, it4 * 4 : (it4 + 1) * 4, h, :], in_=o4[:]
                        )

                # ---- epilogue per token tile ----
                final = p_fin.tile([128, NT, 128], f32, tag="final")
                for it in range(NT):
                    recips = p_ep.tile([128, H, 1], f32, tag="recips")
                    nc.vector.reciprocal(recips[:, :, 0], o_all[:, it, :, D])
                    onorm = p_ep.tile([128, 128], f32, tag="onorm")
                    nc.vector.tensor_mul(
                        onorm[:].rearrange("p (h d) -> p h d", h=H),
                        o_all[:, it, :, 0:D],
                        recips[:].broadcast_to([128, H, D]),
                    )
                    pst2 = ps_o.tile([128, 512], f32, tag="ps_o")
                    nc.tensor.transpose(pst2[:, :128], onorm[:], ident[:])
                    onormT = p_ep.tile([128, 128], f32, tag="onormT")
                    nc.vector.tensor_copy(out=onormT[:], in_=pst2[:, :128])
                    psf = ps_o.tile([128, 512], f32, tag="ps_o")
                    nc.tensor.matmul(
                        psf[:, :128], onormT[:], wout_sb[:], start=True, stop=True
                    )
                    nc.vector.tensor_add(final[:, it, :], psf[:, :128], bout_sb[:])

                nc.sync.dma_start(
                    out.ap()[b].rearrange("(t p) d -> p t d", p=128), final[:]
                )

            def emit_attention_D(b, qkT, vp):
                # heads grouped by q/k tile (A: 0-3, B: 4-7); per group the
                # PV product accumulates transposed: oT[32*hh + r, i] for
                # r<17 (16 dims + softmax-denominator row from the ones col)
                oT_sb = {}
                for qg in range(2):
                    qT = qkT["qA"] if qg == 0 else qkT["qB"]
                    kT = qkT["kA"] if qg == 0 else qkT["kB"]
                    ots = [
                        ps_oe.tile([128, 512], f32, tag="ps_oe", name=f"ot{ih}")
                        for ih in range(2)
                    ]
                    for hh in range(4):
                        h = qg * 4 + hh
                        base = 32 * hh
                        sl = slice(base, base + D)
                        PT = p_PT.tile([128, NT, N], bf16, tag="PT")
                        for c in range(NT):
                            pss = ps_dots.tile([128, 1024], f32, tag="ps_dots")
                            for g in range(N // 512):
                                nc.tensor.matmul(
                                    pss[:, g * 512 : (g + 1) * 512],
                                    kT[sl, c * 128 : (c + 1) * 128],
                                    qT[sl, g * 512 : (g + 1) * 512],
                                    start=True,
                                    stop=True,
                                    tile_position=(base, 0),
                                )
                            nc.scalar.activation(
                                PT[:, c, :],
                                pss[:],
                                mybir.ActivationFunctionType.Exp,
                                bias=mb_sb[:, b, c : c + 1],
                                scale=SCALE,
                            )
                        for c in range(NT):
                            for ih in range(2):
                                nc.tensor.matmul(
                                    ots[ih][base : base + 32, :],
                                    vp[:, c, h, :],
                                    PT[:, c, ih * 512 : (ih + 1) * 512],
                                    start=(c == 0),
                                    stop=(c == NT - 1),
                                    tile_position=(0, base),
                                )
                    for ih in range(2):
                        t = p_oall.tile([128, 512], f32, tag="oTsb", bufs=8)
                        nc.vector.tensor_copy(out=t[:], in_=ots[ih][:])
                        oT_sb[(qg, ih)] = t

                # ---- epilogue per token tile ----
                final = p_fin.tile([128, NT, 128], f32, tag="final")
                for it in range(NT):
                    ih, j = divmod(it, NT // 2)
                    off = j * 128
                    o_t = []
                    for qg in range(2):
                        pt_ps = ps_oe.tile([128, 4, 32], f32, tag="ps_oe")
                        nc.tensor.transpose(
                            pt_ps[:],
                            oT_sb[(qg, ih)][:, off : off + 128],
                            ident[:],
                        )
                        o_t.append(pt_ps)
                    recips = p_ep.tile([128, H], f32, tag="recips")
                    for qg in range(2):
                        nc.vector.reciprocal(
                            recips[:, qg * 4 : (qg + 1) * 4], o_t[qg][:, :, D]
                        )
                    onorm = p_ep.tile([128, 128], f32, tag="onorm")
                    for h in range(H):
                        qg, hh = divmod(h, 4)
                        nc.vector.tensor_scalar_mul(
                            onorm[:, h * D : (h + 1) * D],
                            o_t[qg][:, hh, 0:D],
                            recips[:, h : h + 1],
                        )
                    pst2 = ps_oe.tile([128, 512], f32, tag="ps_oe")
                    nc.tensor.transpose(pst2[:, :128], onorm[:], ident[:])
                    onormT = p_ep.tile([128, 128], f32, tag="onormT")
                    nc.vector.tensor_copy(out=onormT[:], in_=pst2[:, :128])
                    psf = ps_oe.tile([128, 512], f32, tag="ps_oe")
                    nc.tensor.matmul(
                        psf[:, :128], onormT[:], wout_sb[:], start=True, stop=True
                    )
                    nc.vector.tensor_add(final[:, it, :], psf[:, :128], bout_sb[:])

                nc.sync.dma_start(
                    out.ap()[b].rearrange("(t p) d -> p t d", p=128), final[:]
                )

            def prologue_x(b):
                # load x[b], transpose tiles -> xT [dim, tok] bf16
                xin = p_xin.tile([128, NT, 128], f32, tag="xin", name="xin")
                xsrc = x.ap()[b].rearrange("(t p) d -> p t d", p=128)
                nc.sync.dma_start(xin[:, 0 : NT // 2, :], xsrc[:, 0 : NT // 2, :])
                nc.sync.dma_start(xin[:, NT // 2 :, :], xsrc[:, NT // 2 :, :])
                xT = p_xT.tile([128, N], bf16, tag="xT", name="xT")
                for t in range(NT):
                    pst = ps_sm.tile([128, 512], f32, tag="ps_sm", name="pst")
                    nc.tensor.transpose(pst[:, :128], xin[:, t, :], ident[:])
                    nc.vector.tensor_copy(
                        out=xT[:, t * 128 : (t + 1) * 128], in_=pst[:, :128]
                    )
                return xT

            def prologue_qk(b, xT):
                qkT = {}
                for nm, w_sb in (
                    ("qA", wqA_sb),
                    ("kA", wkA_sb),
                    ("qB", wqB_sb),
                    ("kB", wkB_sb),
                ):
                    dst = p_qk.tile([128, N], bf16, tag=nm, name=nm)
                    qkT[nm] = dst
                    for g in range(N // 512):
                        psq = ps_sm.tile([128, 512], f32, tag="ps_sm", name="psq")
                        nc.tensor.matmul(
                            psq[:],
                            w_sb[:],
                            xT[:, g * 512 : (g + 1) * 512],
                            start=True,
                            stop=True,
                        )
                        nc.vector.tensor_copy(
                            out=dst[:, g * 512 : (g + 1) * 512], in_=psq[:]
                        )
                return qkT

            def prologue_v(b, xT):
                vp = p_vp.tile([128, NT, H, VPW], bf16, tag="vp", name="vp")
                for c in range(NT):
                    psv = ps_sm.tile([128, 512], f32, tag="ps_sm", name="psv")
                    nc.tensor.matmul(
                        psv[:, : H * VPW],
                        xT[:, c * 128 : (c + 1) * 128],
                        wvp_sb[:],
                        start=True,
                        stop=True,
                    )
                    nc.vector.tensor_copy(out=vp[:, c], in_=psv[:, : H * VPW])
                    nc.gpsimd.memset(vp[:, c, :, D], 1.0)
                return vp

            def attnv_group(o_all, vp, PT, h, it):
                # one full PV accumulation for token tile `it` of head h
                o1 = ps_o.tile([128, D + 1], f32, tag="ps_o", name="o1")
                for c2 in range(NT):
                    nc.tensor.matmul(
                        o1[:],
                        PT[:, c2, it * 128 : (it + 1) * 128],
                        vp[:, c2, h, 0 : D + 1],
                        start=(c2 == 0),
                        stop=(c2 == NT - 1),
                    )
                nc.vector.tensor_copy(out=o_all[:, it, h, :], in_=o1[:])

            def emit_heads(b, qkT, vp, head_end_filler=None):
                o_all = p_oall.tile(
                    [128, NT, H, D + 1], f32, tag="oall", name="o_all"
                )
                prevPT = None
                for h in range(H):
                    base = 32 * (h % 4)
                    sl = slice(base, base + D)
                    qT = qkT["qA"] if h < 4 else qkT["qB"]
                    kT = qkT["kA"] if h < 4 else qkT["kB"]
                    PT = p_PT.tile([128, NT, N], bf16, tag="PT", name="PT")
                    for c in range(NT):
                        pss = ps_dots.tile(
                            [128, 1024], f32, tag="ps_dots", name="pss"
                        )
                        for g in range(N // 512):
                            nc.tensor.matmul(
                                pss[:, g * 512 : (g + 1) * 512],
                                kT[sl, c * 128 : (c + 1) * 128],
                                qT[sl, g * 512 : (g + 1) * 512],
                                start=True,
                                stop=True,
                                tile_position=(base, 0),
                            )
                        nc.scalar.activation(
                            PT[:, c, :],
                            pss[:],
                            mybir.ActivationFunctionType.Exp,
                            bias=mb_sb[:, b, c : c + 1],
                            scale=SCALE,
                        )
                        if prevPT is not None:
                            attnv_group(o_all, vp, prevPT, h - 1, c)
                    if head_end_filler is not None:
                        head_end_filler(h)
                    prevPT = PT
                return o_all, prevPT

            def emit_epilogue_it(o_all, final, it, tail=False):
                eppool, eptag, epw = (
                    (ps_dots, "ps_dots", 1024) if tail else (ps_o, "ps_o", 512)
                )
                recips = p_ep.tile([128, H, 1], f32, tag="recips", name="recips")
                nc.vector.reciprocal(recips[:, :, 0], o_all[:, it, :, D])
                onorm = p_ep.tile([128, 128], f32, tag="onorm", name="onorm")
                nc.vector.tensor_mul(
                    onorm[:].rearrange("p (h d) -> p h d", h=H),
                    o_all[:, it, :, 0:D],
                    recips[:].broadcast_to([128, H, D]),
                )
                pst2 = eppool.tile([128, epw], f32, tag=eptag, name="pst2")
                nc.tensor.transpose(pst2[:, :128], onorm[:], ident[:])
                onormT = p_ep.tile([128, 128], f32, tag="onormT", name="onormT")
                nc.vector.tensor_copy(out=onormT[:], in_=pst2[:, :128])
                psf = eppool.tile([128, epw], f32, tag=eptag, name="psf")
                nc.tensor.matmul(
                    psf[:, :128], onormT[:], wout_sb[:], start=True, stop=True
                )
                nc.vector.tensor_add(final[:, it, :], psf[:, :128], bout_sb[:])

            def emit_body_E():
                finals = [
                    p_fin.tile([128, NT, 128], f32, tag="final", name=f"final{b}")
                    for b in range(BLOC)
                ]
                xT0 = prologue_x(0)
                qkT0 = prologue_qk(0, xT0)
                vp0 = prologue_v(0, xT0)
                st1 = {}

                def filler0(h):
                    # build batch 1's inputs while batch 0's heads stream
                    if h == 0:
                        st1["xT"] = prologue_x(1)
                    elif h == 1:
                        st1["qkT"] = prologue_qk(1, st1["xT"])
                    elif h == 2:
                        st1["vp"] = prologue_v(1, st1["xT"])

                oall0, lastPT0 = emit_heads(0, qkT0, vp0, filler0)

                def filler1(h):
                    if h == 0:
                        for it in range(NT):
                            attnv_group(oall0, vp0, lastPT0, H - 1, it)
                    if h >= 2:
                        emit_epilogue_it(oall0, finals[0], h - 2)

                oall1, lastPT1 = emit_heads(
                    1, st1["qkT"], st1["vp"], head_end_filler=filler1
                )
                for it in (NT - 2, NT - 1):
                    emit_epilogue_it(oall0, finals[0], it, tail=True)
                nc.sync.dma_start(
                    out.ap()[0].rearrange("(t p) d -> p t d", p=128), finals[0][:]
                )
                out1 = out.ap()[1].rearrange("(t p) d -> p t d", p=128)
                for it in range(NT):
                    attnv_group(oall1, st1["vp"], lastPT1, H - 1, it)
                    emit_epilogue_it(oall1, finals[1], it, tail=True)
                    nc.sync.dma_start(out1[:, it, :], finals[1][:, it, :])

            def emit_all():
                if variant == "E":
                    emit_body_E()
                else:
                    for b in range(BLOC):
                        emit_batch(b)

            if reps == 1:
                emit_all()
            else:
                # on-device loop: one dispatch runs the body `reps` times
                # (used for wall-clock-marginal timing measurements)
                with tc.For_i(0, reps, 1):
                    emit_all()

    nc.compile()
    return nc


def _get_program(reps=1, variant=None):
    key = ("nc", reps, variant or VARIANT)
    if key not in _cache:
        _cache[key] = _build_program(reps, variant)
    return _cache[key]


def _host_prep(x, mask, maps, Wqkv, Wout, bout):
    """Build per-core input maps (weight repacking + mask bias precompute)."""
    import ml_dtypes

    bfd = ml_dtypes.bfloat16
    x = np.ascontiguousarray(np.asarray(x, np.float32))
    Wqkv = np.asarray(Wqkv, np.float32)
    Wout = np.ascontiguousarray(np.asarray(Wout, np.float32))
    bout = np.asarray(bout, np.float32)
    Wq, Wk, Wv = Wqkv[:, :DIM], Wqkv[:, DIM : 2 * DIM], Wqkv[:, 2 * DIM :]

    def pack_qk(W, hs):
        out = np.zeros((DIM, 128), np.float32)
        for q, h in enumerate(hs):
            out[:, 32 * q : 32 * q + D] = W[:, D * h : D * (h + 1)]
        return out

    wqA = pack_qk(Wq, range(0, 4)).astype(bfd)
    wqB = pack_qk(Wq, range(4, 8)).astype(bfd)
    wkA = pack_qk(Wk, range(0, 4)).astype(bfd)
    wkB = pack_qk(Wk, range(4, 8)).astype(bfd)
    wvp = np.zeros((DIM, H * 32), np.float32)
    for h in range(H):
        wvp[:, 32 * h : 32 * h + D] = Wv[:, D * h : D * (h + 1)]
    wvp = wvp.astype(bfd)
    boutB = np.broadcast_to(bout, (128, DIM)).copy()

    # combined key mask (block mask broadcasts over the full batch: B//K^2 == 1)
    m = np.concatenate([np.ones((1, 1), np.float32), np.asarray(mask, np.float32)], 1)
    mp = np.concatenate(
        [np.ones((B, 1), np.float32), np.asarray(maps, np.float32)], 1
    )
    keep = m * mp  # [B, N]
    mbias = ((keep - 1.0) * (-MASK_BIAS)).astype(np.float32)  # 0 / -300
    # [B, N] -> [B, chunk, 128] -> [B, 128, chunk]
    mbias = mbias.reshape(B, NT, 128).transpose(0, 2, 1).copy()

    in_maps = []
    for i in range(NCORES):
        in_maps.append(
            {
                "x": x[BLOC * i : BLOC * (i + 1)],
                "wqA": wqA,
                "wqB": wqB,
                "wkA": wkA,
                "wkB": wkB,
                "wvp": wvp,
                "wout": Wout,
                "boutB": boutB,
                "mb": np.ascontiguousarray(
                    mbias[BLOC * i : BLOC * (i + 1)]
                ),
            }
        )
    return in_maps


class _Runner:
    """Cached SPMD runner: builds the sharded PJRT executable once so
    repeated kernel() calls skip re-trace / re-compile / NEFF reload."""

    def __init__(self, nc, n_cores):
        import jax
        from jax.sharding import Mesh, PartitionSpec
        from jax.experimental.shard_map import shard_map
        import concourse.mybir as mybir
        from concourse import bass2jax
        from concourse.bass2jax import _bass_exec_p, install_neuronx_cc_hook

        install_neuronx_cc_hook()
        self.jax = jax
        self.n_cores = n_cores
        in_names, out_names, out_avals, zero_outs = [], [], [], []
        partition_name = (
            nc.partition_id_tensor.name if nc.partition_id_tensor else None
        )
        for alloc in nc.m.functions[0].allocations:
            if not isinstance(alloc, mybir.MemoryLocationSet):
                continue
            name = alloc.memorylocations[0].name
            if alloc.kind == "ExternalInput":
                if name != partition_name:
                    in_names.append(name)
            elif alloc.kind == "ExternalOutput":
                shape = tuple(alloc.tensor_shape)
                dtype = mybir.dt.np(alloc.dtype)
                out_names.append(name)
                out_avals.append(jax.core.ShapedArray(shape, dtype))
                zero_outs.append(np.zeros(shape, dtype))
        self.in_names, self.out_names = in_names, out_names
        self.out_avals, self.zero_outs = out_avals, zero_outs
        n_params = len(in_names)
        all_in = list(in_names) + list(out_names)
        if partition_name is not None:
            all_in.append(partition_name)

        def _body(*args):
            operands = list(args)
            if partition_name is not None:
                operands.append(bass2jax.partition_id_tensor())
            return tuple(
                _bass_exec_p.bind(
                    *operands,
                    out_avals=tuple(out_avals),
                    in_names=tuple(all_in),
                    out_names=tuple(out_names),
                    lowering_input_output_aliases=(),
                    sim_require_finite=True,
                    sim_require_nnan=True,
                    nc=nc,
                )
            )

        devices = jax.devices()[:n_cores]
        mesh = Mesh(np.asarray(devices), ("core",))
        n_outs = len(out_names)
        self.fn = jax.jit(
            shard_map(
                _body,
                mesh=mesh,
                in_specs=(PartitionSpec("core"),) * (n_params + n_outs),
                out_specs=(PartitionSpec("core"),) * n_outs,
                check_rep=False,
            ),
            keep_unused=True,
        )

    def run(self, in_maps):
        n = self.n_cores
        args = [
            np.concatenate([np.asarray(in_maps[c][nm]) for c in range(n)], 0)
            for nm in self.in_names
        ] + [
            np.zeros((n * z.shape[0], *z.shape[1:]), z.dtype)
            for z in self.zero_outs
        ]
        outs = self.fn(*args)
        self.jax.block_until_ready(outs)
        return [
            {
                nm: np.asarray(outs[i]).reshape(n, *self.out_avals[i].shape)[c]
                for i, nm in enumerate(self.out_names)
            }
            for c in range(n)
        ]


def _get_runner():
    if "runner" not in _cache:
        _cache["runner"] = _Runner(_get_program(), NCORES)
    return _cache["runner"]


def kernel(x, mask, maps, Wqkv, Wout, bout, K):
    in_maps = _host_prep(x, mask, maps, Wqkv, Wout, bout)
    results = _get_runner().run(in_maps)
    return np.concatenate(
        [results[i]["out"] for i in range(NCORES)], axis=0
    ).astype(np.float32)


# revision 30
# speedup vs baseline: 1.2406x; 1.0241x over previous
"""Trainium2 Bass kernel for nn_Attention_16518444221223 (sparse_attention).

Strategy: data-parallel over batch (16 seqs -> 8 cores x 2 seqs). Per core a
flash-attention-style kernel that never materializes the [b,h,n,n] score
tensor in HBM:
  - x tiles are PE-transposed once; QKV projections run in fp32 on the PE.
  - q/k are stored transposed ([d, tok]) in bf16 with the 8 heads (d=16)
    packed at partition offsets {0,32,64,96} across two tiles (the PE
    requires K<=32 operands to sit at 32-aligned base partitions).
  - scores are computed transposed (S^T[j, i] chunks), so the key-position
    mask folds into the per-partition bias of the Exp activation:
        P^T = Exp(scale * S^T + bias[j]),  bias = 0 (keep) / -300 (masked)
    No max-subtraction is needed: scaled dots are within +-5 for this
    problem, exp() cannot overflow, and softmax is shift-invariant.
  - P^T is the bf16 *stationary* operand of the PV matmul (fast weight
    load), with V carrying an extra ones-column per head so the softmax
    denominator falls out of the same matmul.
  - normalize, transpose via PE, output-project in fp32, add bias, DMA out.
"""

import sys

sys.path.insert(0, "/opt/trn_rl_repo")

import numpy as np

B, N, DIM, H, D = 16, 1024, 128, 8, 16
NCORES = 8
BLOC = B // NCORES  # 2 sequences per core
SCALE = float(DIM) ** -0.5
MASK_BIAS = -300.0
NT = N // 128  # 8 token tiles per sequence

_cache = {}
VARIANT = "E"  # "A": 2-bank dots psum + single [128,1024] exp per chunk
               # "B": 1-bank dots psum + two [128,512] exps per chunk
               # "D": like A, but attnV uses moving-P matmuls (8x fewer PE
               #      instructions; o comes out transposed, epilogue adjusts)
               # "E": phase-batched (both seqs prologue -> heads -> epilogues)
               #      with PV matmuls interleaved into the dots stream


def _build_program(reps=1, variant=None):
    if variant is None:
        variant = VARIANT
    import concourse.mybir as mybir
    import concourse.tile as tile
    from concourse import bacc
    from concourse._compat import axon_active
    from concourse.masks import make_identity

    f32 = mybir.dt.float32
    bf16 = mybir.dt.bfloat16

    nc = bacc.Bacc(
        "TRN2",
        target_bir_lowering=False,
        debug=not axon_active(),
        num_devices=NCORES,
    )

    xt = nc.dram_tensor("xt", [BLOC, DIM, N], bf16, kind="ExternalInput")
    wqA = nc.dram_tensor("wqA", [DIM, 128], bf16, kind="ExternalInput")
    wqB = nc.dram_tensor("wqB", [DIM, 128], bf16, kind="ExternalInput")
    wkA = nc.dram_tensor("wkA", [DIM, 128], bf16, kind="ExternalInput")
    wkB = nc.dram_tensor("wkB", [DIM, 128], bf16, kind="ExternalInput")
    VPW = 32  # per-head V block width: 16 dims + 1 ones-col + 15 zero pad
    wvp = nc.dram_tensor("wvp", [DIM, H * VPW], bf16, kind="ExternalInput")
    wout = nc.dram_tensor("wout", [DIM, DIM], f32, kind="ExternalInput")
    boutB = nc.dram_tensor("boutB", [128, DIM], f32, kind="ExternalInput")
    mb = nc.dram_tensor("mb", [BLOC, 128, NT], f32, kind="ExternalInput")
    out = nc.dram_tensor("out", [BLOC, N, DIM], f32, kind="ExternalOutput")

    with tile.TileContext(nc) as tc:
        with (
            tc.tile_pool(name="consts", bufs=1) as consts,
            tc.tile_pool(name="xin", bufs=2) as p_xin,
            tc.tile_pool(name="xT", bufs=2) as p_xT,
            tc.tile_pool(name="qk", bufs=2) as p_qk,
            tc.tile_pool(name="vp", bufs=2) as p_vp,
            tc.tile_pool(name="PT", bufs=3) as p_PT,
            tc.tile_pool(name="oall", bufs=2) as p_oall,
            tc.tile_pool(name="ep", bufs=3) as p_ep,
            tc.tile_pool(name="fin", bufs=2) as p_fin,
            tc.tile_pool(
                name="ps_dots", bufs=(4 if variant == "B" else 2), space="PSUM"
            ) as ps_dots,
            tc.tile_pool(name="ps_sm", bufs=2, space="PSUM") as ps_sm,
            tc.tile_pool(name="ps_o", bufs=2, space="PSUM") as ps_o,
            tc.tile_pool(name="ps_oe", bufs=2, space="PSUM") as ps_oe,
        ):
            # PSUM banks: A: dots 2x2 + prologue 2 + (o/epilogue) 2 = 8
            #             B: dots 4x1 + prologue 2 + (o/epilogue) 2 = 8
            # ---- constants ----
            wqA_sb = consts.tile([128, 128], bf16, tag="wqA")
            wqB_sb = consts.tile([128, 128], bf16, tag="wqB")
            wkA_sb = consts.tile([128, 128], bf16, tag="wkA")
            wkB_sb = consts.tile([128, 128], bf16, tag="wkB")
            wvp_sb = consts.tile([128, H * VPW], bf16, tag="wvp")
            wout_sb = consts.tile([128, 128], f32, tag="wout")
            bout_sb = consts.tile([128, 128], f32, tag="boutB")
            mb_sb = consts.tile([128, BLOC, NT], f32, tag="mb")
            ident = consts.tile([128, 128], f32, tag="ident")

            nc.sync.dma_start(wqA_sb[:], wqA.ap())
            nc.sync.dma_start(wkA_sb[:], wkA.ap())
            nc.sync.dma_start(
                mb_sb[:], mb.ap().rearrange("b p t -> p b t")
            )
            nc.sync.dma_start(wqB_sb[:], wqB.ap())
            nc.sync.dma_start(wkB_sb[:], wkB.ap())
            nc.sync.dma_start(wvp_sb[:], wvp.ap())
            nc.sync.dma_start(wout_sb[:], wout.ap())
            nc.sync.dma_start(bout_sb[:], boutB.ap())
            make_identity(nc, ident[:])
            actwarm = consts.tile([128, 1], f32, tag="actwarm")
            nc.gpsimd.memset(actwarm[:], 0.0)
            nc.scalar.activation(
                actwarm[:], actwarm[:], mybir.ActivationFunctionType.Exp
            )

            def emit_batch(b):
                # ---- load x[b], transpose tiles -> xT_b [dim, tok] ----
                xin = p_xin.tile([128, NT, 128], f32, tag="xin")
                nc.sync.dma_start(
                    xin[:], x.ap()[b].rearrange("(t p) d -> p t d", p=128)
                )
                xT = p_xT.tile([128, N], bf16, tag="xT")
                for t in range(NT):
                    pst = ps_sm.tile([128, 512], f32, tag="ps_sm")
                    nc.tensor.transpose(pst[:, :128], xin[:, t, :], ident[:])
                    nc.vector.tensor_copy(
                        out=xT[:, t * 128 : (t + 1) * 128], in_=pst[:, :128]
                    )

                # ---- q/k projections (fp32 PE) -> bf16 transposed layouts ----
                qkT = {}
                for nm, w_sb in (
                    ("qA", wqA_sb),
                    ("kA", wkA_sb),
                    ("qB", wqB_sb),
                    ("kB", wkB_sb),
                ):
                    dst = p_qk.tile([128, N], bf16, tag=nm)
                    qkT[nm] = dst
                    for g in range(N // 512):
                        psq = ps_sm.tile([128, 512], f32, tag="ps_sm")
                        nc.tensor.matmul(
                            psq[:],
                            w_sb[:],
                            xT[:, g * 512 : (g + 1) * 512],
                            start=True,
                            stop=True,
                        )
                        nc.vector.tensor_copy(
                            out=dst[:, g * 512 : (g + 1) * 512], in_=psq[:]
                        )

                # ---- v projection -> vp_b [128, chunk, head, 17] bf16 ----
                vp = p_vp.tile([128, NT, H, VPW], bf16, tag="vp")
                for c in range(NT):
                    psv = ps_sm.tile([128, 512], f32, tag="ps_sm")
                    nc.tensor.matmul(
                        psv[:, : H * VPW],
                        xT[:, c * 128 : (c + 1) * 128],
                        wvp_sb[:],
                        start=True,
                        stop=True,
                    )
                    nc.vector.tensor_copy(out=vp[:, c], in_=psv[:, : H * VPW])
                    nc.gpsimd.memset(vp[:, c, :, D], 1.0)

                # ---- attention ----
                if variant == "D":
                    emit_attention_D(b, qkT, vp)
                    return
                o_all = p_oall.tile([128, NT, H, D + 1], f32, tag="oall")
                for h in range(H):
                    base = 32 * (h % 4)
                    sl = slice(base, base + D)
                    qT = qkT["qA"] if h < 4 else qkT["qB"]
                    kT = qkT["kA"] if h < 4 else qkT["kB"]
                    PT = p_PT.tile([128, NT, N], bf16, tag="PT")
                    for c in range(NT):
                        if variant == "A":
                            pss = ps_dots.tile([128, 1024], f32, tag="ps_dots")
                            for g in range(N // 512):
                                nc.tensor.matmul(
                                    pss[:, g * 512 : (g + 1) * 512],
                                    kT[sl, c * 128 : (c + 1) * 128],
                                    qT[sl, g * 512 : (g + 1) * 512],
                                    start=True,
                                    stop=True,
                                    tile_position=(base, 0),
                                )
                            nc.scalar.activation(
                                PT[:, c, :],
                                pss[:],
                                mybir.ActivationFunctionType.Exp,
                                bias=mb_sb[:, b, c : c + 1],
                                scale=SCALE,
                            )
                        else:
                            for g in range(N // 512):
                                pss = ps_dots.tile([128, 512], f32, tag="ps_dots")
                                nc.tensor.matmul(
                                    pss[:],
                                    kT[sl, c * 128 : (c + 1) * 128],
                                    qT[sl, g * 512 : (g + 1) * 512],
                                    start=True,
                                    stop=True,
                                    tile_position=(base, 0),
                                )
                                nc.scalar.activation(
                                    PT[:, c, g * 512 : (g + 1) * 512],
                                    pss[:],
                                    mybir.ActivationFunctionType.Exp,
                                    bias=mb_sb[:, b, c : c + 1],
                                    scale=SCALE,
                                )
                    for it4 in range(NT // 4):
                        o4 = ps_o.tile([128, 4, D + 1], f32, tag="ps_o", name="o4")
                        for itm in range(4):
                            it = it4 * 4 + itm
                            for c in range(NT):
                                nc.tensor.matmul(
                                    o4[:, itm, :],
                                    PT[:, c, it * 128 : (it + 1) * 128],
                                    vp[:, c, h, 0 : D + 1],
                                    start=(c == 0),
                                    stop=(c == NT - 1),
                                )
                        nc.vector.tensor_copy(
                            out=o_all[:, it4 * 4 : (it4 + 1) * 4, h, :], in_=o4[:]
                        )

                # ---- epilogue per token tile ----
                final = p_fin.tile([128, NT, 128], f32, tag="final")
                for it in range(NT):
                    recips = p_ep.tile([128, H, 1], f32, tag="recips")
                    nc.vector.reciprocal(recips[:, :, 0], o_all[:, it, :, D])
                    onorm = p_ep.tile([128, 128], f32, tag="onorm")
                    nc.vector.tensor_mul(
                        onorm[:].rearrange("p (h d) -> p h d", h=H),
                        o_all[:, it, :, 0:D],
                        recips[:].broadcast_to([128, H, D]),
                    )
                    pst2 = ps_o.tile([128, 512], f32, tag="ps_o")
                    nc.tensor.transpose(pst2[:, :128], onorm[:], ident[:])
                    onormT = p_ep.tile([128, 128], f32, tag="onormT")
                    nc.vector.tensor_copy(out=onormT[:], in_=pst2[:, :128])
                    psf = ps_o.tile([128, 512], f32, tag="ps_o")
                    nc.tensor.matmul(
                        psf[:, :128], onormT[:], wout_sb[:], start=True, stop=True
                    )
                    nc.vector.tensor_add(final[:, it, :], psf[:, :128], bout_sb[:])

                nc.sync.dma_start(
                    out.ap()[b].rearrange("(t p) d -> p t d", p=128), final[:]
                )

            def emit_attention_D(b, qkT, vp):
                # heads grouped by q/k tile (A: 0-3, B: 4-7); per group the
                # PV product accumulates transposed: oT[32*hh + r, i] for
                # r<17 (16 dims + softmax-denominator row from the ones col)
                oT_sb = {}
                for qg in range(2):
                    qT = qkT["qA"] if qg == 0 else qkT["qB"]
                    kT = qkT["kA"] if qg == 0 else qkT["kB"]
                    ots = [
                        ps_oe.tile([128, 512], f32, tag="ps_oe", name=f"ot{ih}")
                        for ih in range(2)
                    ]
                    for hh in range(4):
                        h = qg * 4 + hh
                        base = 32 * hh
                        sl = slice(base, base + D)
                        PT = p_PT.tile([128, NT, N], bf16, tag="PT")
                        for c in range(NT):
                            pss = ps_dots.tile([128, 1024], f32, tag="ps_dots")
                            for g in range(N // 512):
                                nc.tensor.matmul(
                                    pss[:, g * 512 : (g + 1) * 512],
                                    kT[sl, c * 128 : (c + 1) * 128],
                                    qT[sl, g * 512 : (g + 1) * 512],
                                    start=True,
                                    stop=True,
                                    tile_position=(base, 0),
                                )
                            nc.scalar.activation(
                                PT[:, c, :],
                                pss[:],
                                mybir.ActivationFunctionType.Exp,
                                bias=mb_sb[:, b, c : c + 1],
                                scale=SCALE,
                            )
                        for c in range(NT):
                            for ih in range(2):
                                nc.tensor.matmul(
                                    ots[ih][base : base + 32, :],
                                    vp[:, c, h, :],
                                    PT[:, c, ih * 512 : (ih + 1) * 512],
                                    start=(c == 0),
                                    stop=(c == NT - 1),
                                    tile_position=(0, base),
                                )
                    for ih in range(2):
                        t = p_oall.tile([128, 512], f32, tag="oTsb", bufs=8)
                        nc.vector.tensor_copy(out=t[:], in_=ots[ih][:])
                        oT_sb[(qg, ih)] = t

                # ---- epilogue per token tile ----
                final = p_fin.tile([128, NT, 128], f32, tag="final")
                for it in range(NT):
                    ih, j = divmod(it, NT // 2)
                    off = j * 128
                    o_t = []
                    for qg in range(2):
                        pt_ps = ps_oe.tile([128, 4, 32], f32, tag="ps_oe")
                        nc.tensor.transpose(
                            pt_ps[:],
                            oT_sb[(qg, ih)][:, off : off + 128],
                            ident[:],
                        )
                        o_t.append(pt_ps)
                    recips = p_ep.tile([128, H], f32, tag="recips")
                    for qg in range(2):
                        nc.vector.reciprocal(
                            recips[:, qg * 4 : (qg + 1) * 4], o_t[qg][:, :, D]
                        )
                    onorm = p_ep.tile([128, 128], f32, tag="onorm")
                    for h in range(H):
                        qg, hh = divmod(h, 4)
                        nc.vector.tensor_scalar_mul(
                            onorm[:, h * D : (h + 1) * D],
                            o_t[qg][:, hh, 0:D],
                            recips[:, h : h + 1],
                        )
                    pst2 = ps_oe.tile([128, 512], f32, tag="ps_oe")
                    nc.tensor.transpose(pst2[:, :128], onorm[:], ident[:])
                    onormT = p_ep.tile([128, 128], f32, tag="onormT")
                    nc.vector.tensor_copy(out=onormT[:], in_=pst2[:, :128])
                    psf = ps_oe.tile([128, 512], f32, tag="ps_oe")
                    nc.tensor.matmul(
                        psf[:, :128], onormT[:], wout_sb[:], start=True, stop=True
                    )
                    nc.vector.tensor_add(final[:, it, :], psf[:, :128], bout_sb[:])

                nc.sync.dma_start(
                    out.ap()[b].rearrange("(t p) d -> p t d", p=128), final[:]
                )

            def prologue_x(b):
                # x arrives host-transposed (dim-major) in bf16: straight DMA
                xT = p_xT.tile([128, N], bf16, tag="xT", name="xT")
                nc.sync.dma_start(xT[:, 0 : N // 2], xt.ap()[b][:, 0 : N // 2])
                nc.sync.dma_start(xT[:, N // 2 :], xt.ap()[b][:, N // 2 :])
                return xT

            def prologue_qk(b, xT):
                qkT = {}
                for nm, w_sb in (
                    ("qA", wqA_sb),
                    ("kA", wkA_sb),
                    ("qB", wqB_sb),
                    ("kB", wkB_sb),
                ):
                    dst = p_qk.tile([128, N], bf16, tag=nm, name=nm)
                    qkT[nm] = dst
                    for g in range(N // 512):
                        psq = ps_sm.tile([128, 512], f32, tag="ps_sm", name="psq")
                        nc.tensor.matmul(
                            psq[:],
                            w_sb[:],
                            xT[:, g * 512 : (g + 1) * 512],
                            start=True,
                            stop=True,
                        )
                        nc.vector.tensor_copy(
                            out=dst[:, g * 512 : (g + 1) * 512], in_=psq[:]
                        )
                return qkT

            def prologue_v(b, xT):
                vp = p_vp.tile([128, NT, H, VPW], bf16, tag="vp", name="vp")
                for c in range(NT):
                    psv = ps_sm.tile([128, 512], f32, tag="ps_sm", name="psv")
                    nc.tensor.matmul(
                        psv[:, : H * VPW],
                        xT[:, c * 128 : (c + 1) * 128],
                        wvp_sb[:],
                        start=True,
                        stop=True,
                    )
                    nc.vector.tensor_copy(out=vp[:, c], in_=psv[:, : H * VPW])
                    nc.gpsimd.memset(vp[:, c, :, D], 1.0)
                return vp

            def attnv_group(o_all, vp, PT, h, it):
                # one full PV accumulation for token tile `it` of head h
                o1 = ps_o.tile([128, D + 1], f32, tag="ps_o", name="o1")
                for c2 in range(NT):
                    nc.tensor.matmul(
                        o1[:],
                        PT[:, c2, it * 128 : (it + 1) * 128],
                        vp[:, c2, h, 0 : D + 1],
                        start=(c2 == 0),
                        stop=(c2 == NT - 1),
                    )
                nc.vector.tensor_copy(out=o_all[:, it, h, :], in_=o1[:])

            def emit_heads(b, qkT, vp, head_end_filler=None):
                o_all = p_oall.tile(
                    [128, NT, H, D + 1], f32, tag="oall", name="o_all"
                )
                prevPT = None
                for h in range(H):
                    base = 32 * (h % 4)
                    sl = slice(base, base + D)
                    qT = qkT["qA"] if h < 4 else qkT["qB"]
                    kT = qkT["kA"] if h < 4 else qkT["kB"]
                    PT = p_PT.tile([128, NT, N], bf16, tag="PT", name="PT")
                    for c in range(NT):
                        pss = ps_dots.tile(
                            [128, 1024], f32, tag="ps_dots", name="pss"
                        )
                        for g in range(N // 512):
                            nc.tensor.matmul(
                                pss[:, g * 512 : (g + 1) * 512],
                                kT[sl, c * 128 : (c + 1) * 128],
                                qT[sl, g * 512 : (g + 1) * 512],
                                start=True,
                                stop=True,
                                tile_position=(base, 0),
                            )
                        nc.scalar.activation(
                            PT[:, c, :],
                            pss[:],
                            mybir.ActivationFunctionType.Exp,
                            bias=mb_sb[:, b, c : c + 1],
                            scale=SCALE,
                        )
                        if prevPT is not None:
                            attnv_group(o_all, vp, prevPT, h - 1, c)
                    if head_end_filler is not None:
                        head_end_filler(h)
                    prevPT = PT
                return o_all, prevPT

            def emit_epilogue_it(o_all, final, it, tail=False):
                eppool, eptag, epw = (
                    (ps_dots, "ps_dots", 1024) if tail else (ps_o, "ps_o", 512)
                )
                recips = p_ep.tile([128, H, 1], f32, tag="recips", name="recips")
                nc.vector.reciprocal(recips[:, :, 0], o_all[:, it, :, D])
                onorm = p_ep.tile([128, 128], f32, tag="onorm", name="onorm")
                nc.vector.tensor_mul(
                    onorm[:].rearrange("p (h d) -> p h d", h=H),
                    o_all[:, it, :, 0:D],
                    recips[:].broadcast_to([128, H, D]),
                )
                pst2 = eppool.tile([128, epw], f32, tag=eptag, name="pst2")
                nc.tensor.transpose(pst2[:, :128], onorm[:], ident[:])
                onormT = p_ep.tile([128, 128], f32, tag="onormT", name="onormT")
                nc.vector.tensor_copy(out=onormT[:], in_=pst2[:, :128])
                psf = eppool.tile([128, epw], f32, tag=eptag, name="psf")
                nc.tensor.matmul(
                    psf[:, :128], onormT[:], wout_sb[:], start=True, stop=True
                )
                nc.vector.tensor_add(final[:, it, :], psf[:, :128], bout_sb[:])

            def emit_body_E():
                finals = [
                    p_fin.tile([128, NT, 128], f32, tag="final", name=f"final{b}")
                    for b in range(BLOC)
                ]
                xT0 = prologue_x(0)
                qkT0 = prologue_qk(0, xT0)
                vp0 = prologue_v(0, xT0)
                st1 = {}

                def filler0(h):
                    # build batch 1's inputs while batch 0's heads stream
                    if h == 0:
                        st1["xT"] = prologue_x(1)
                    elif h == 1:
                        st1["qkT"] = prologue_qk(1, st1["xT"])
                    elif h == 2:
                        st1["vp"] = prologue_v(1, st1["xT"])

                oall0, lastPT0 = emit_heads(0, qkT0, vp0, filler0)

                def filler1(h):
                    if h == 0:
                        for it in range(NT):
                            attnv_group(oall0, vp0, lastPT0, H - 1, it)
                    if h >= 2:
                        emit_epilogue_it(oall0, finals[0], h - 2)

                oall1, lastPT1 = emit_heads(
                    1, st1["qkT"], st1["vp"], head_end_filler=filler1
                )
                for it in (NT - 2, NT - 1):
                    emit_epilogue_it(oall0, finals[0], it, tail=True)
                nc.sync.dma_start(
                    out.ap()[0].rearrange("(t p) d -> p t d", p=128), finals[0][:]
                )
                out1 = out.ap()[1].rearrange("(t p) d -> p t d", p=128)
                for it in range(NT):
                    attnv_group(oall1, st1["vp"], lastPT1, H - 1, it)
                    emit_epilogue_it(oall1, finals[1], it, tail=True)
                    nc.sync.dma_start(out1[:, it, :], finals[1][:, it, :])

            def emit_all():
                if variant == "E":
                    emit_body_E()
                else:
                    for b in range(BLOC):
                        emit_batch(b)

            if reps == 1:
                emit_all()
            else:
                # on-device loop: one dispatch runs the body `reps` times
                # (used for wall-clock-marginal timing measurements)
                with tc.For_i(0, reps, 1):
                    emit_all()

    nc.compile()
    return nc


def _get_program(reps=1, variant=None):
    key = ("nc", reps, variant or VARIANT)
    if key not in _cache:
        _cache[key] = _build_program(reps, variant)
    return _cache[key]


def _host_prep(x, mask, maps, Wqkv, Wout, bout):
    """Build per-core input maps (weight repacking + mask bias precompute)."""
    import ml_dtypes

    bfd = ml_dtypes.bfloat16
    xtp = np.ascontiguousarray(
        np.asarray(x, np.float32).transpose(0, 2, 1)
    ).astype(bfd)
    Wqkv = np.asarray(Wqkv, np.float32)
    Wout = np.ascontiguousarray(np.asarray(Wout, np.float32))
    bout = np.asarray(bout, np.float32)
    Wq, Wk, Wv = Wqkv[:, :DIM], Wqkv[:, DIM : 2 * DIM], Wqkv[:, 2 * DIM :]

    def pack_qk(W, hs):
        out = np.zeros((DIM, 128), np.float32)
        for q, h in enumerate(hs):
            out[:, 32 * q : 32 * q + D] = W[:, D * h : D * (h + 1)]
        return out

    wqA = pack_qk(Wq, range(0, 4)).astype(bfd)
    wqB = pack_qk(Wq, range(4, 8)).astype(bfd)
    wkA = pack_qk(Wk, range(0, 4)).astype(bfd)
    wkB = pack_qk(Wk, range(4, 8)).astype(bfd)
    wvp = np.zeros((DIM, H * 32), np.float32)
    for h in range(H):
        wvp[:, 32 * h : 32 * h + D] = Wv[:, D * h : D * (h + 1)]
    wvp = wvp.astype(bfd)
    boutB = np.broadcast_to(bout, (128, DIM)).copy()

    # combined key mask (block mask broadcasts over the full batch: B//K^2 == 1)
    m = np.concatenate([np.ones((1, 1), np.float32), np.asarray(mask, np.float32)], 1)
    mp = np.concatenate(
        [np.ones((B, 1), np.float32), np.asarray(maps, np.float32)], 1
    )
    keep = m * mp  # [B, N]
    mbias = ((keep - 1.0) * (-MASK_BIAS)).astype(np.float32)  # 0 / -300
    # [B, N] -> [B, chunk, 128] -> [B, 128, chunk]
    mbias = mbias.reshape(B, NT, 128).transpose(0, 2, 1).copy()

    in_maps = []
    for i in range(NCORES):
        in_maps.append(
            {
                "xt": xtp[BLOC * i : BLOC * (i + 1)],
                "wqA": wqA,
                "wqB": wqB,
                "wkA": wkA,
                "wkB": wkB,
                "wvp": wvp,
                "wout": Wout,
                "boutB": boutB,
                "mb": np.ascontiguousarray(
                    mbias[BLOC * i : BLOC * (i + 1)]
                ),
            }
        )
    return in_maps


class _Runner:
    """Cached SPMD runner: builds the sharded PJRT executable once so
    repeated kernel() calls skip re-trace / re-compile / NEFF reload."""

    def __init__(self, nc, n_cores):
        import jax
        from jax.sharding import Mesh, PartitionSpec
        from jax.experimental.shard_map import shard_map
        import concourse.mybir as mybir
        from concourse import bass2jax
        from concourse.bass2jax import _bass_exec_p, install_neuronx_cc_hook

        install_neuronx_cc_hook()
        self.jax = jax
        self.n_cores = n_cores
        in_names, out_names, out_avals, zero_outs = [], [], [], []
        partition_name = (
            nc.partition_id_tensor.name if nc.partition_id_tensor else None
        )
        for alloc in nc.m.functions[0].allocations:
            if not isinstance(alloc, mybir.MemoryLocationSet):
                continue
            name = alloc.memorylocations[0].name
            if alloc.kind == "ExternalInput":
                if name != partition_name:
                    in_names.append(name)
            elif alloc.kind == "ExternalOutput":
                shape = tuple(alloc.tensor_shape)
                dtype = mybir.dt.np(alloc.dtype)
                out_names.append(name)
                out_avals.append(jax.core.ShapedArray(shape, dtype))
                zero_outs.append(np.zeros(shape, dtype))
        self.in_names, self.out_names = in_names, out_names
        self.out_avals, self.zero_outs = out_avals, zero_outs
        n_params = len(in_names)
        all_in = list(in_names) + list(out_names)
        if partition_name is not None:
            all_in.append(partition_name)

        def _body(*args):
            operands = list(args)
            if partition_name is not None:
                operands.append(bass2jax.partition_id_tensor())
            return tuple(
                _bass_exec_p.bind(
                    *operands,
                    out_avals=tuple(out_avals),
                    in_names=tuple(all_in),
                    out_names=tuple(out_names),
                    lowering_input_output_aliases=(),
                    sim_require_finite=True,
                    sim_require_nnan=True,
                    nc=nc,
                )
            )

        devices = jax.devices()[:n_cores]
        mesh = Mesh(np.asarray(devices), ("core",))
        n_outs = len(out_names)
        self.fn = jax.jit(
            shard_map(
                _body,
                mesh=mesh,
                in_specs=(PartitionSpec("core"),) * (n_params + n_outs),
                out_specs=(PartitionSpec("core"),) * n_outs,
                check_rep=False,
            ),
            keep_unused=True,
        )

    def run(self, in_maps):
        n = self.n_cores
        args = [
            np.concatenate([np.asarray(in_maps[c][nm]) for c in range(n)], 0)
            for nm in self.in_names
        ] + [
            np.zeros((n * z.shape[0], *z.shape[1:]), z.dtype)
            for z in self.zero_outs
        ]
        outs = self.fn(*args)
        self.jax.block_until_ready(outs)
        return [
            {
                nm: np.asarray(outs[i]).reshape(n, *self.out_avals[i].shape)[c]
                for i, nm in enumerate(self.out_names)
            }
            for c in range(n)
        ]


def _get_runner():
    if "runner" not in _cache:
        _cache["runner"] = _Runner(_get_program(), NCORES)
    return _cache["runner"]


def kernel(x, mask, maps, Wqkv, Wout, bout, K):
    in_maps = _host_prep(x, mask, maps, Wqkv, Wout, bout)
    results = _get_runner().run(in_maps)
    return np.concatenate(
        [results[i]["out"] for i in range(NCORES)], axis=0
    ).astype(np.float32)
